# revision 4
# baseline (speedup 1.0000x reference)
"""JointSRLModule kernel for 8 Trainium2 NeuronCores.

Data-parallel over batch B=8: one batch row per core via jax.pmap; the
per-row module is expressed with Trainium-friendly primitives only
(matmul / compare / reductions / exp / relu) — no sort, top_k, gather or
scatter HLOs, which the trn2 tensorizer rejects or lowers poorly:

- all index gathers -> one-hot selection matmuls (exact in fp32)
- attentive span pooling -> separable softmax:
    att[n,w] = logits[end_n - w] so with t' = t the unnormalized weight of
    token t in span n is exp(logits[t]) * V[n,t], V an interval mask
    [max(start_n, end_n-15, 0) <= t <= end_n]; normalize by the row sum.
- top-k -> rank via pairwise comparison counts (ties broken by index),
  kept set = rank < num_keep, fill = max index with rank < K,
  ascending compaction via cumsum (tril matmul) + one-hot matmul.
"""
import os
os.environ.setdefault("NEURON_CC_FLAGS", "--auto-cast=none")

import numpy as np
import jax
import jax.numpy as jnp

B, T, H = 8, 512, 1024
NA, NP = 1024, 512
NW, WD = 16, 20
NCLS = 66
K_ARG, K_PRED = 30, 15
NEG = -1e20


def _onehot_f(idx, depth):
    # idx [N] int32 -> [N, depth] f32 one-hot (iota compare)
    return (idx[:, None] == jnp.arange(depth, dtype=idx.dtype)[None, :]).astype(jnp.float32)


def _rank_desc(s):
    # rank of each element in descending order, ties -> lower index first
    n = s.shape[0]
    gt = (s[None, :] > s[:, None]).astype(jnp.float32)          # [n, n] m beats n
    eq = (s[None, :] == s[:, None]).astype(jnp.float32)
    earlier = (jnp.arange(n)[None, :] < jnp.arange(n)[:, None]).astype(jnp.float32)
    return gt.sum(-1) + (eq * earlier).sum(-1)                   # [n] f32


def _prune_idx(scores, num_keep, K):
    """scores [N] -> (sel [K, N] one-hot selection of sorted top idx, top_mask-ish kept flags [K])."""
    n = scores.shape[0]
    r = _rank_desc(scores)                                       # [N]
    kept = (r < num_keep).astype(jnp.float32)                    # [N] exactly num_keep ones
    topk = (r < K).astype(jnp.float32)
    idxf = jnp.arange(n, dtype=jnp.float32)
    fill = jnp.max(topk * idxf - (1.0 - topk))                   # max index among top-K
    # ascending compaction: slot of kept n = (# kept m <= n) - 1
    tril = (jnp.arange(n)[None, :] <= jnp.arange(n)[:, None]).astype(jnp.float32)
    c = tril @ kept                                              # inclusive cumsum [N]
    slot = c - 1.0
    ks = jnp.arange(K, dtype=jnp.float32)
    sel = (slot[None, :] == ks[:, None]).astype(jnp.float32) * kept[None, :]   # [K, N]
    pad = (ks[:, None] >= num_keep).astype(jnp.float32)          # rows that are fill
    fill_oh = (jnp.arange(n) == fill.astype(jnp.int32)).astype(jnp.float32)
    sel = sel * (1.0 - pad) + pad * fill_oh[None, :]
    keep_flags = (ks < num_keep).astype(jnp.float32)             # [K]
    return sel, keep_flags


def _row(features, arg_candidates, predicate_candidates, W_width, W_att, b_att,
         Wa1, ba1, Wa2, ba2, Wp1, bp1, Wp2, bp2, Ws1, bs1, Ws2, bs2):
    F = features                                                 # [T, H]
    a_s, a_e = arg_candidates[:, 0], arg_candidates[:, 1]        # [NA] int32
    p_s, p_e = predicate_candidates[:, 0], predicate_candidates[:, 1]
    arg_mask = (a_e > 0).astype(jnp.float32)
    pred_mask = (p_e > 0).astype(jnp.float32)

    # ---- endpoint embeddings via one-hot matmuls (masked indices) ----
    am_i = arg_mask.astype(a_s.dtype)
    pm_i = pred_mask.astype(p_s.dtype)
    a_sm, a_em = a_s * am_i, a_e * am_i
    p_sm, p_em = p_s * pm_i, p_e * pm_i

    Gs_a = _onehot_f(a_sm, T)                                    # [NA, T]
    Ge_a = _onehot_f(a_em, T)
    Gs_p = _onehot_f(p_sm, T)                                    # [NP, T]
    Ge_p = _onehot_f(p_em, T)
    a_w = jnp.clip(a_em - a_sm, 0, NW - 1)
    Gw_a = _onehot_f(a_w, NW)                                    # [NA, NW]

    ep_start = Gs_a @ F                                          # [NA, H]
    ep_end = Ge_a @ F
    ep_width = Gw_a @ W_width                                    # [NA, WD]

    # ---- attentive via separable softmax ----
    logits = (F @ W_att + b_att)[:, 0]                           # [T]
    E = jnp.exp(logits)                                          # [T] |logits| small
    lo = jnp.maximum(jnp.maximum(a_s, a_e - (NW - 1)), 0)        # [NA]
    hi = a_e
    t = jnp.arange(T, dtype=a_s.dtype)
    V = ((t[None, :] >= lo[:, None]) & (t[None, :] <= hi[:, None])).astype(jnp.float32)
    Wmat = V * E[None, :]                                        # [NA, T]
    Z = Wmat.sum(-1)
    A = Wmat / jnp.maximum(Z, 1e-30)[:, None]
    attended = A @ F                                             # [NA, H]

    mask_col = arg_mask[:, None]
    arg_emb = jnp.concatenate([ep_start, ep_end, ep_width, attended], -1) * mask_col

    # ---- arg scores + prune ----
    h = jax.nn.relu(arg_emb @ Wa1 + ba1)
    scores_a = (h @ Wa2 + ba2)                                   # [NA, 1]
    scores_a = jnp.where(mask_col > 0, scores_a, NEG)
    n_keep_arg = jnp.minimum((arg_mask.sum() * 0.8).astype(jnp.int32), K_ARG)
    sel_a, keepf_a = _prune_idx(scores_a[:, 0], n_keep_arg, K_ARG)   # [K_ARG, NA]
    top_arg_emb = sel_a @ arg_emb                                # [K, 3H+WD]
    top_arg_scores = sel_a @ scores_a                            # [K, 1]
    top_arg_mask = keepf_a * ((sel_a @ arg_mask[:, None])[:, 0] > 0)
    top_arg_spans = (sel_a @ arg_candidates.astype(jnp.float32)).astype(arg_candidates.dtype)

    # ---- pred embeddings + prune ----
    pred_emb = jnp.concatenate([Gs_p @ F, Ge_p @ F], -1) * pred_mask[:, None]
    hp = jax.nn.relu(pred_emb @ Wp1 + bp1)
    scores_p = (hp @ Wp2 + bp2)
    scores_p = jnp.where(pred_mask[:, None] > 0, scores_p, NEG)
    n_keep_pred = jnp.minimum((pred_mask.sum() * 0.4).astype(jnp.int32), K_PRED)
    sel_p, keepf_p = _prune_idx(scores_p[:, 0], n_keep_pred, K_PRED)
    top_pred_emb = sel_p @ pred_emb
    top_pred_scores = sel_p @ scores_p
    top_pred_mask = keepf_p * ((sel_p @ pred_mask[:, None])[:, 0] > 0)
    top_pred_spans = (sel_p @ predicate_candidates.astype(jnp.float32)).astype(predicate_candidates.dtype)

    # ---- pairwise scorer ----
    dp = top_pred_emb.shape[-1]
    pred_proj = top_pred_emb @ Ws1[:dp]                          # [Kp, H]
    arg_proj = top_arg_emb @ Ws1[dp:]                            # [Ka, H]
    hb = jax.nn.relu(pred_proj[:, None, :] + arg_proj[None, :, :] + bs1)
    scores = hb @ Ws2 + bs2                                      # [Kp, Ka, NCLS-1]
    scores = scores + top_arg_scores[None, :, :] + top_pred_scores[:, None, :]
    dummy = jnp.zeros(scores.shape[:-1] + (1,), scores.dtype)
    srl_scores = jnp.concatenate([dummy, scores], -1)

    return (srl_scores, top_pred_spans, top_arg_spans, top_pred_mask.astype(jnp.float32),
            top_arg_mask.astype(jnp.float32), scores_p, scores_a)


_PMAPPED = None


def _get_pmapped():
    global _PMAPPED
    if _PMAPPED is None:
        _PMAPPED = jax.pmap(
            _row,
            in_axes=(0, 0, 0) + (None,) * 15,
            devices=jax.devices()[:8],
        )
    return _PMAPPED


def kernel(features, arg_candidates, predicate_candidates, W_width, W_att, b_att,
           Wa1, ba1, Wa2, ba2, Wp1, bp1, Wp2, bp2, Ws1, bs1, Ws2, bs2):
    span_dtype = np.asarray(arg_candidates).dtype
    f = _get_pmapped()
    args32 = np.asarray(arg_candidates).astype(np.int32)
    preds32 = np.asarray(predicate_candidates).astype(np.int32)
    out = f(jnp.asarray(np.asarray(features, dtype=np.float32)),
            jnp.asarray(args32), jnp.asarray(preds32),
            *[jnp.asarray(np.asarray(x, dtype=np.float32)) for x in
              (W_width, W_att, b_att, Wa1, ba1, Wa2, ba2, Wp1, bp1, Wp2, bp2,
               Ws1, bs1, Ws2, bs2)])
    (srl, tps, tas, tpm, tam, pfs, afs) = [np.asarray(o) for o in out]
    return (srl.astype(np.float32), tps.astype(span_dtype), tas.astype(span_dtype),
            tpm.astype(np.float32), tam.astype(np.float32),
            pfs.astype(np.float32), afs.astype(np.float32))
